# revision 6
# baseline (speedup 1.0000x reference)
"""Trainium2 Bass kernel for nn_Decoder_91122026151952.

Math (reference collapses because LSTMCell state is zero every step):
    gates = x @ W_ih.T + (b_ih + b_hh)        # h0 == 0, W_hh unused
    i, f, g, o = split(gates, 4)              # f unused (c_prev == 0)
    c = sigmoid(i) * tanh(g)
    h = sigmoid(o) * tanh(c)                  # [B, T, H]
    out = softmax((h.reshape(B, T*H) @ W_out.T + b_out).reshape(B, 4, 10), -1)

Device formulation (all-sigmoid, fp16 operands, fp32 accumulation):
    tanh(z) = 2*sigmoid(2z) - 1, with the *2 on g folded into the weights:
    AGO = sigmoid(x_aug @ W1aug)   where W1aug = [Wi.T | 2*Wg.T | Wo.T] plus a
    bias row matched to a ones-channel appended to x.
    c = 2*A*G - A ; S = sigmoid(2c) ; h = 2*O*S - O
    logits.T [40, B] accumulate on PE over 43200-deep contraction, bias via a
    rank-1 ones matmul, final transpose + softmax on-chip.

Sharding: pure data parallel over batch (1024 -> 8 x 128).
Host prep: shard/cast/transpose of inputs only (x -> [H+pad, T, B_c] fp16).
"""

import numpy as np

B, T, H, OUT = 1024, 240, 180, 40
NCORES = 8
BC = B // NCORES            # 128 batches per core
G3 = 3 * H                  # 540 gate columns (i, 2g, o)
HP = 256                    # padded H: 0..179 data, 180 ones, 181..255 zero
TB = 16                     # timesteps per input DMA batch
SB = 8                      # timesteps per DVE/ACT super-batch
PB = 2                      # timesteps per PSUM gates batch
SEG = 64                    # timesteps per matmul2 PSUM accumulation segment

_CACHE = {}


def _build():
    import concourse.bass as bass
    import concourse.tile as tile
    from concourse import mybir

    f16 = mybir.dt.float16
    f32 = mybir.dt.float32
    ALU = mybir.AluOpType
    ACTF = mybir.ActivationFunctionType

    nc = bass.Bass("TRN2")

    xT = nc.dram_tensor("xT", [HP, T, BC], f16, kind="ExternalInput")
    w1 = nc.dram_tensor("w1", [H + 1, G3], f16, kind="ExternalInput")
    w2hi = nc.dram_tensor("w2hi", [128, T * OUT], f16, kind="ExternalInput")
    w2lo = nc.dram_tensor("w2lo", [H - 128, T * OUT], f16, kind="ExternalInput")
    bout = nc.dram_tensor("bout", [1, OUT], f16, kind="ExternalInput")
    eye = nc.dram_tensor("eye", [OUT, OUT], f32, kind="ExternalInput")
    y = nc.dram_tensor("y", [BC, OUT], f32, kind="ExternalOutput")

    with tile.TileContext(nc) as tc:
        with (
            tc.tile_pool(name="consts", bufs=1) as consts,
            tc.tile_pool(name="xtiles", bufs=3) as xtiles,
            tc.tile_pool(name="work", bufs=3) as work,
            tc.tile_pool(name="htp", bufs=6) as htp,
            tc.tile_pool(name="gpsum", bufs=2, space="PSUM") as gpsum,
            tc.tile_pool(name="mpsum", bufs=2, space="PSUM") as mpsum,
        ):
            # ---- constants ----
            w1hi = consts.tile([128, G3], f16)
            nc.sync.dma_start(out=w1hi, in_=w1[0:128, :])
            w1lo = consts.tile([H + 1 - 128, G3], f16)           # 53 rows
            nc.sync.dma_start(out=w1lo, in_=w1[128 : H + 1, :])
            w2hi_sb = consts.tile([128, T * OUT], f16)
            nc.sync.dma_start(out=w2hi_sb, in_=w2hi[:, :])
            w2lo_sb = consts.tile([H - 128, T * OUT], f16)       # 52 rows
            nc.sync.dma_start(out=w2lo_sb, in_=w2lo[:, :])
            bout_sb = consts.tile([1, OUT], f16)
            nc.sync.dma_start(out=bout_sb, in_=bout[:, :])
            eye_sb = consts.tile([OUT, OUT], f32)
            nc.sync.dma_start(out=eye_sb, in_=eye[:, :])
            ones_sb = consts.tile([1, BC], f16)
            nc.vector.memset(ones_sb, 1.0)
            facc = consts.tile([OUT, BC], f32)
            nc.vector.memset(facc, 0.0)

            n_seg = (T + SEG - 1) // SEG
            mm2_slots = []

            xthi = xtlo = None
            ago = hbuf = None
            first_in_seg = False

            for t in range(T):
                # ---- input loads, TB timesteps at a time ----
                tb, ti = divmod(t, TB)
                if ti == 0:
                    xthi = xtiles.tile([128, TB, BC], f16, tag="xthi")
                    nc.sync.dma_start(
                        out=xthi, in_=xT[0:128, t : t + TB, :]
                    )
                    xtlo = xtiles.tile([H + 1 - 128, TB, BC], f16, tag="xtlo")
                    nc.sync.dma_start(
                        out=xtlo, in_=xT[128 : H + 1, t : t + TB, :]
                    )

                sb, si = divmod(t, SB)
                if si == 0:
                    ago = work.tile([128, 3, SB, H], f16, tag="ago")
                    # cols H..HP are never written: the lo-transpose DMA moves
                    # those junk bytes into htlo rows 52.., which nothing reads
                    hbuf = work.tile([128, SB, HP], f16, tag="hbuf")

                seg, segi = divmod(t, SEG)
                if segi == 0:
                    mm2_ps = mpsum.tile([OUT, BC], f32, tag="mm2")
                    mm2_slots.append(mm2_ps)
                    first_in_seg = True
                    if seg == 0:
                        # bias as the opening rank-1 accumulation
                        nc.tensor.matmul(
                            mm2_ps, bout_sb, ones_sb,
                            start=True, stop=False, skip_group_check=True,
                        )
                        first_in_seg = False

                # ---- matmul1: gates for this timestep ----
                pb, pi = divmod(t, PB)
                if pi == 0:
                    gps = gpsum.tile([128, 3, PB, 256], f32, tag="gates")
                for gate in range(3):
                    nc.tensor.matmul(
                        gps[:, gate, pi, 0:H],
                        xthi[:, ti, :],
                        w1hi[:, gate * H : (gate + 1) * H],
                        start=True, stop=False,
                    )
                for gate in range(3):
                    nc.tensor.matmul(
                        gps[:, gate, pi, 0:H],
                        xtlo[:, ti, :],
                        w1lo[:, gate * H : (gate + 1) * H],
                        start=False, stop=True,
                    )

                # ---- sigmoid over the PB-batch of gates (PSUM -> SBUF) ----
                if pi == PB - 1:
                    nc.scalar.activation(
                        out=ago[:, :, si - (PB - 1) : si + 1, :],
                        in_=gps[:, :, :, 0:H],
                        func=ACTF.Sigmoid,
                    )

                # ---- DVE fixups + second sigmoid + h, per SB super-batch ----
                if si == SB - 1:
                    tmp = work.tile([128, SB, H], f16, tag="tmp")
                    # tmp = (A * 2) * G
                    nc.vector.scalar_tensor_tensor(
                        tmp, ago[:, 0], 2.0, ago[:, 1],
                        op0=ALU.mult, op1=ALU.mult,
                    )
                    cbuf = work.tile([128, SB, H], f16, tag="cbuf")
                    # c = 2AG - A
                    nc.vector.tensor_tensor(cbuf, tmp, ago[:, 0], op=ALU.subtract)
                    sbuf_s = work.tile([128, SB, H], f16, tag="sbuf_s")
                    # S = sigmoid(2c)
                    nc.scalar.activation(
                        out=sbuf_s, in_=cbuf, func=ACTF.Sigmoid, scale=2.0
                    )
                    tmp2 = work.tile([128, SB, H], f16, tag="tmp2")
                    # tmp2 = (O * 2) * S
                    nc.vector.scalar_tensor_tensor(
                        tmp2, ago[:, 2], 2.0, sbuf_s,
                        op0=ALU.mult, op1=ALU.mult,
                    )
                    # h = 2OS - O
                    nc.vector.tensor_tensor(
                        hbuf[:, :, 0:H], tmp2, ago[:, 2], op=ALU.subtract
                    )

                    # ---- transpose h and accumulate matmul2 ----
                    for k in range(SB):
                        tk = sb * SB + k
                        hthi = htp.tile([128, BC], f16, tag="hthi")
                        nc.sync.dma_start(
                            out=hthi, in_=hbuf[:, k, 0:128], transpose=True
                        )
                        htlo = htp.tile([128, BC], f16, tag="htlo")
                        nc.sync.dma_start(
                            out=htlo, in_=hbuf[:, k, 128:256], transpose=True
                        )
                        nc.tensor.matmul(
                            mm2_ps,
                            w2hi_sb[:, tk * OUT : (tk + 1) * OUT],
                            hthi,
                            start=first_in_seg, stop=False,
                            skip_group_check=True,
                        )
                        first_in_seg = False
                        last = (tk % SEG == SEG - 1) or (tk == T - 1)
                        nc.tensor.matmul(
                            mm2_ps,
                            w2lo_sb[:, tk * OUT : (tk + 1) * OUT],
                            htlo[0 : H - 128, :],
                            start=False, stop=last,
                            skip_group_check=True,
                        )
                        if last:
                            nc.vector.tensor_tensor(
                                facc, facc, mm2_ps, op=ALU.add
                            )

            # ---- tail: transpose logits, softmax ----
            tr_ps = gpsum.tile([BC, OUT], f32, tag="gates")
            nc.tensor.transpose(tr_ps, facc, eye_sb)
            e_sb = consts.tile([BC, OUT], f32)
            nc.scalar.activation(out=e_sb, in_=tr_ps, func=ACTF.Exp)
            ssum = consts.tile([BC, 4], f32)
            nc.vector.tensor_reduce(
                ssum,
                e_sb.rearrange("p (g k) -> p g k", g=4),
                axis=mybir.AxisListType.X,
                op=ALU.add,
            )
            rinv = consts.tile([BC, 4], f32)
            nc.vector.reciprocal(rinv, ssum)
            y_sb = consts.tile([BC, OUT], f32)
            for g in range(4):
                nc.vector.tensor_scalar(
                    y_sb[:, g * 10 : (g + 1) * 10],
                    e_sb[:, g * 10 : (g + 1) * 10],
                    rinv[:, g : g + 1],
                    None,
                    op0=ALU.mult,
                )
            nc.sync.dma_start(out=y[:, :], in_=y_sb)

    _split_excess_waits(nc)
    return nc


def _split_excess_waits(nc):
    """walrus' per-instruction ISA structs have fewer sync-wait slots than
    Tile sometimes emits ("Too many sync wait commands"). For any instruction
    carrying >1 wait, insert EventSemaphore wait-carriers (one wait each)
    immediately before it on the same engine queue. The sequencer blocks on
    those first, then on the instruction's remaining wait — semantics are
    identical, no reordering is introduced."""
    import bass_rust
    import concourse.mybir as mybir

    n_new = 0
    for f in nc.m.functions:
        for blk in f.blocks:
            il = blk.instructions
            idx = 0
            while idx < len(il):
                ins = il[idx]
                si = getattr(ins, "sync_info", None)
                eng = getattr(ins, "engine", None)
                waits = list(si.on_wait) if si is not None else []
                if len(waits) >= 2 and eng is not None:
                    for w in waits[:-1]:
                        ev = mybir.InstEventSemaphore(
                            name=f"EVW-{n_new}", ins=[], outs=[]
                        )
                        n_new += 1
                        ev.engine = eng
                        ev.sync_info = bass_rust.SyncInfo(
                            on_wait=[w], on_update=[]
                        )
                        il.insert(idx, ev)
                        idx += 1
                    ins.sync_info = bass_rust.SyncInfo(
                        on_wait=[waits[-1]], on_update=list(si.on_update)
                    )
                idx += 1


def _prep_inputs(x, W_ih, b_ih, b_hh, W_out, b_out):
    """Host-side sharding prep: cast/transpose/augment. Returns per-core maps."""
    f16 = np.float16
    b = (b_ih + b_hh).astype(np.float32)
    Wi, Wg, Wo = W_ih[0:H], W_ih[2 * H : 3 * H], W_ih[3 * H : 4 * H]
    bi, bg, bo = b[0:H], b[2 * H : 3 * H], b[3 * H : 4 * H]
    W1 = np.concatenate([Wi.T, 2.0 * Wg.T, Wo.T], axis=1).astype(np.float32)
    brow = np.concatenate([bi, 2.0 * bg, bo])[None, :]
    w1a = np.ascontiguousarray(
        np.concatenate([W1, brow], axis=0), dtype=np.float32
    ).astype(f16)                                            # [181, 540]

    # W_out [40, 43200] -> [t, h, o] -> partition-major halves [p, t*o]
    w2 = W_out.reshape(OUT, T, H).transpose(1, 2, 0).astype(f16)  # [T, H, OUT]
    w2hi = np.ascontiguousarray(w2[:, 0:128, :].transpose(1, 0, 2)).reshape(
        128, T * OUT
    )
    w2lo = np.ascontiguousarray(w2[:, 128:H, :].transpose(1, 0, 2)).reshape(
        H - 128, T * OUT
    )

    boutq = b_out.astype(f16)[None, :]                       # [1, 40]
    eye = np.eye(OUT, dtype=np.float32)

    # x -> per-core [HP, T, BC] fp16 with ones channel at row H
    xs = x.reshape(NCORES, BC, T, H).astype(f16)
    in_maps = []
    for c in range(NCORES):
        xc = np.zeros((HP, T, BC), dtype=f16)
        xc[0:H] = xs[c].transpose(2, 1, 0)                   # [H, T, BC]
        xc[H] = 1.0
        in_maps.append(
            {
                "xT": np.ascontiguousarray(xc),
                "w1": w1a,
                "w2hi": w2hi,
                "w2lo": w2lo,
                "bout": boutq,
                "eye": eye,
            }
        )
    return in_maps


def kernel(x, W_ih, W_hh, b_ih, b_hh, W_out, b_out, _bench=None):
    x = np.asarray(x, dtype=np.float32)
    W_ih = np.asarray(W_ih, dtype=np.float32)
    b_ih = np.asarray(b_ih, dtype=np.float32)
    b_hh = np.asarray(b_hh, dtype=np.float32)
    W_out = np.asarray(W_out, dtype=np.float32)
    b_out = np.asarray(b_out, dtype=np.float32)

    from concourse.bass_utils import run_bass_kernel_spmd

    if "nc" not in _CACHE:
        _CACHE["nc"] = _build()
    nc = _CACHE["nc"]

    in_maps = _prep_inputs(x, W_ih, b_ih, b_hh, W_out, b_out)
    kwargs = dict(_bench) if _bench else {}
    res = run_bass_kernel_spmd(nc, in_maps, core_ids=list(range(NCORES)), **kwargs)
    out = np.concatenate([r["y"] for r in res.results], axis=0)  # [1024, 40]
    if _bench is not None:
        _CACHE["last_result"] = res
    return out.reshape(B, 4, 10).astype(np.float32)


# revision 7
# speedup vs baseline: 2.0709x; 2.0709x over previous
"""Trainium2 Bass kernel for nn_Decoder_91122026151952.

Math (reference collapses because LSTMCell state is zero every step):
    gates = x @ W_ih.T + (b_ih + b_hh)        # h0 == 0, W_hh unused
    i, f, g, o = split(gates, 4)              # f unused (c_prev == 0)
    c = sigmoid(i) * tanh(g)
    h = sigmoid(o) * tanh(c)                  # [B, T, H]
    out = softmax((h.reshape(B, T*H) @ W_out.T + b_out).reshape(B, 4, 10), -1)

Device formulation (all-sigmoid, fp16 operands, fp32 accumulation):
    tanh(z) = 2*sigmoid(2z) - 1, with the *2 on g folded into the weights:
    AGO = sigmoid(x_aug @ W1aug)   where W1aug = [Wi.T | 2*Wg.T | Wo.T] plus a
    bias row matched to a ones-channel appended to x.
    S  = sigmoid(4 * A * (G - 0.5))           # == sigmoid(2c)
    h' = O * (S - 0.5)                        # == h/2; W_out doubled on host
    logits.T [40, B] accumulate on PE over a flat 43264-deep contraction
    (h' transposed on the DMA xbar in 1024-column chunks), bias via a rank-1
    ones matmul, final PE transpose + softmax on-chip.

Sharding: pure data parallel over batch (1024 -> 8 x 128).
Host prep: shard/cast/transpose/augment of inputs only.
"""

import numpy as np

B, T, H, OUT = 1024, 240, 180, 40
NCORES = 8
BC = B // NCORES            # 128 batches per core
G3 = 3 * H                  # 540 gate columns (i, 2g, o)
TB = 16                     # timesteps per input DMA batch
SB = 8                      # timesteps per DVE/ACT super-batch
PB = 2                      # timesteps per PSUM gates batch
TH = T * H                  # 43200 contraction depth of matmul2
THP = 43264                 # padded to a multiple of 128 (338 chunks)
NCH = THP // 128            # 338 th-slices
XCH = 1024                  # h-ring columns per xbar transpose chunk

_CACHE = {}


def _build():
    import concourse.bass as bass
    import concourse.tile as tile
    from concourse import mybir

    f16 = mybir.dt.float16
    f32 = mybir.dt.float32
    ALU = mybir.AluOpType
    ACTF = mybir.ActivationFunctionType

    nc = bass.Bass("TRN2")

    xT = nc.dram_tensor("xT", [H + 1, T, BC], f16, kind="ExternalInput")
    w1 = nc.dram_tensor("w1", [H + 1, G3], f16, kind="ExternalInput")
    w2 = nc.dram_tensor("w2", [128, NCH * OUT], f16, kind="ExternalInput")
    bout = nc.dram_tensor("bout", [1, OUT], f16, kind="ExternalInput")
    eye = nc.dram_tensor("eye", [OUT, OUT], f32, kind="ExternalInput")
    y = nc.dram_tensor("y", [BC, OUT], f32, kind="ExternalOutput")

    with tile.TileContext(nc) as tc:
        with (
            tc.tile_pool(name="consts", bufs=1) as consts,
            tc.tile_pool(name="xtiles", bufs=3) as xtiles,
            tc.tile_pool(name="work", bufs=3) as work,
            tc.tile_pool(name="htp", bufs=4) as htp,
            tc.tile_pool(name="gpsum", bufs=2, space="PSUM") as gpsum,
            tc.tile_pool(name="mpsum", bufs=1, space="PSUM") as mpsum,
        ):
            # ---- constants ----
            w1hi = consts.tile([128, G3], f16)
            nc.sync.dma_start(out=w1hi, in_=w1[0:128, :])
            w1lo = consts.tile([H + 1 - 128, G3], f16)           # 53 rows
            nc.sync.dma_start(out=w1lo, in_=w1[128 : H + 1, :])
            w2_sb = consts.tile([128, NCH * OUT], f16)
            nc.sync.dma_start(out=w2_sb, in_=w2[:, :])
            bout_sb = consts.tile([1, OUT], f16)
            nc.sync.dma_start(out=bout_sb, in_=bout[:, :])
            eye_sb = consts.tile([OUT, OUT], f32)
            nc.sync.dma_start(out=eye_sb, in_=eye[:, :])
            ones_sb = consts.tile([1, BC], f16)
            nc.vector.memset(ones_sb, 1.0)
            # resident h' ring over the full T; tail cols padded with zeros
            # (matching zero rows in w2, but must not be NaN garbage)
            hring = consts.tile([128, THP], f16)
            nc.vector.memset(hring[:, TH:THP], 0.0)

            # matmul2 accumulator: one PSUM bank, one accumulation group
            mm2_ps = mpsum.tile([OUT, BC], f32)
            nc.tensor.matmul(
                mm2_ps, bout_sb, ones_sb,
                start=True, stop=False, skip_group_check=True,
            )

            xthi = xtlo = None
            ago = None
            next_chunk = 0

            def emit_chunks(upto):
                """Transpose + matmul2-accumulate all full XCH chunks of the
                h' ring that are complete up to column `upto`."""
                nonlocal next_chunk
                while next_chunk * XCH + XCH <= upto or (
                    upto >= TH and next_chunk * XCH < THP
                ):
                    c0 = next_chunk * XCH
                    c1 = min(c0 + XCH, THP)
                    k = (c1 - c0) // 128
                    htc = htp.tile([128, XCH // 128, 128], f16, tag="htc")
                    nc.sync.dma_start(
                        out=htc[:, :k, :], in_=hring[:, c0:c1], transpose=True
                    )
                    for i in range(k):
                        sl = c0 // 128 + i
                        nc.tensor.matmul(
                            mm2_ps,
                            w2_sb[:, sl * OUT : (sl + 1) * OUT],
                            htc[:, i, :],
                            start=False, stop=(sl == NCH - 1),
                            skip_group_check=True,
                        )
                    next_chunk += 1

            for t in range(T):
                # ---- input loads, TB timesteps at a time ----
                ti = t % TB
                if ti == 0:
                    xthi = xtiles.tile([128, TB, BC], f16, tag="xthi")
                    nc.sync.dma_start(out=xthi, in_=xT[0:128, t : t + TB, :])
                    xtlo = xtiles.tile([H + 1 - 128, TB, BC], f16, tag="xtlo")
                    nc.sync.dma_start(
                        out=xtlo, in_=xT[128 : H + 1, t : t + TB, :]
                    )

                sb, si = divmod(t, SB)
                if si == 0:
                    ago = work.tile([128, 3, SB, H], f16, tag="ago")

                # ---- matmul1: gates for this timestep ----
                pi = t % PB
                if pi == 0:
                    gps = gpsum.tile([128, 3, PB, 256], f32, tag="gates")
                for gate in range(3):
                    nc.tensor.matmul(
                        gps[:, gate, pi, 0:H],
                        xthi[:, ti, :],
                        w1hi[:, gate * H : (gate + 1) * H],
                        start=True, stop=False,
                    )
                for gate in range(3):
                    nc.tensor.matmul(
                        gps[:, gate, pi, 0:H],
                        xtlo[:, ti, :],
                        w1lo[:, gate * H : (gate + 1) * H],
                        start=False, stop=True,
                    )

                # ---- sigmoid over the PB-batch of gates (PSUM -> SBUF) ----
                if pi == PB - 1:
                    nc.scalar.activation(
                        out=ago[:, :, si - (PB - 1) : si + 1, :],
                        in_=gps[:, :, :, 0:H],
                        func=ACTF.Sigmoid,
                    )

                # ---- DVE fixups + second sigmoid + h', per super-batch ----
                if si == SB - 1:
                    g2 = work.tile([128, SB, H], f16, tag="g2")
                    nc.vector.tensor_scalar(
                        g2, ago[:, 1], 0.5, None, op0=ALU.subtract
                    )
                    u = work.tile([128, SB, H], f16, tag="u")
                    nc.vector.tensor_tensor(u, ago[:, 0], g2, op=ALU.mult)
                    sS = work.tile([128, SB, H], f16, tag="sS")
                    nc.scalar.activation(
                        out=sS, in_=u, func=ACTF.Sigmoid, scale=4.0
                    )
                    s2 = work.tile([128, SB, H], f16, tag="s2")
                    nc.vector.tensor_scalar(
                        s2, sS, 0.5, None, op0=ALU.subtract
                    )
                    # h' = O * (S - 0.5) straight into the ring
                    nc.vector.tensor_tensor(
                        hring[:, sb * SB * H : (sb + 1) * SB * H],
                        ago[:, 2].rearrange("p s h -> p (s h)"),
                        s2.rearrange("p s h -> p (s h)"),
                        op=ALU.mult,
                    )
                    emit_chunks((sb + 1) * SB * H)

            # ---- tail: transpose logits, softmax ----
            facc = consts.tile([OUT, BC], f32)
            nc.vector.tensor_copy(facc, mm2_ps)
            tr_ps = gpsum.tile([BC, OUT], f32, tag="gates")
            nc.tensor.transpose(tr_ps, facc, eye_sb)
            e_sb = consts.tile([BC, OUT], f32)
            nc.scalar.activation(out=e_sb, in_=tr_ps, func=ACTF.Exp)
            ssum = consts.tile([BC, 4], f32)
            nc.vector.tensor_reduce(
                ssum,
                e_sb.rearrange("p (g k) -> p g k", g=4),
                axis=mybir.AxisListType.X,
                op=ALU.add,
            )
            rinv = consts.tile([BC, 4], f32)
            nc.vector.reciprocal(rinv, ssum)
            y_sb = consts.tile([BC, OUT], f32)
            for g in range(4):
                nc.vector.tensor_scalar(
                    y_sb[:, g * 10 : (g + 1) * 10],
                    e_sb[:, g * 10 : (g + 1) * 10],
                    rinv[:, g : g + 1],
                    None,
                    op0=ALU.mult,
                )
            nc.sync.dma_start(out=y[:, :], in_=y_sb)

    _split_excess_waits(nc)
    return nc


def _split_excess_waits(nc):
    """walrus' per-instruction ISA structs have fewer sync-wait slots than
    Tile sometimes emits ("Too many sync wait commands"). For any instruction
    carrying >1 wait, insert EventSemaphore wait-carriers (one wait each)
    immediately before it on the same engine queue. The sequencer blocks on
    those first, then on the instruction's remaining wait — semantics are
    identical, no reordering is introduced."""
    import bass_rust
    import concourse.mybir as mybir

    n_new = 0
    for f in nc.m.functions:
        for blk in f.blocks:
            il = blk.instructions
            idx = 0
            while idx < len(il):
                ins = il[idx]
                si = getattr(ins, "sync_info", None)
                eng = getattr(ins, "engine", None)
                waits = list(si.on_wait) if si is not None else []
                if len(waits) >= 2 and eng is not None:
                    for w in waits[:-1]:
                        ev = mybir.InstEventSemaphore(
                            name=f"EVW-{n_new}", ins=[], outs=[]
                        )
                        n_new += 1
                        ev.engine = eng
                        ev.sync_info = bass_rust.SyncInfo(
                            on_wait=[w], on_update=[]
                        )
                        il.insert(idx, ev)
                        idx += 1
                    ins.sync_info = bass_rust.SyncInfo(
                        on_wait=[waits[-1]], on_update=list(si.on_update)
                    )
                idx += 1


def _prep_inputs(x, W_ih, b_ih, b_hh, W_out, b_out):
    """Host-side sharding prep: cast/transpose/augment. Returns per-core maps."""
    f16 = np.float16
    b = (b_ih + b_hh).astype(np.float32)
    Wi, Wg, Wo = W_ih[0:H], W_ih[2 * H : 3 * H], W_ih[3 * H : 4 * H]
    bi, bg, bo = b[0:H], b[2 * H : 3 * H], b[3 * H : 4 * H]
    W1 = np.concatenate([Wi.T, 2.0 * Wg.T, Wo.T], axis=1).astype(np.float32)
    brow = np.concatenate([bi, 2.0 * bg, bo])[None, :]
    w1a = np.ascontiguousarray(
        np.concatenate([W1, brow], axis=0), dtype=np.float32
    ).astype(f16)                                            # [181, 540]

    # W_out [40, 43200] -> x2 (h' = h/2) -> flat th-major, pad, partition-tile
    w2f = np.zeros((THP, OUT), dtype=np.float32)
    w2f[:TH] = 2.0 * W_out.reshape(OUT, TH).T
    w2t = (
        w2f.reshape(NCH, 128, OUT).transpose(1, 0, 2).reshape(128, NCH * OUT)
    ).astype(f16)

    boutq = b_out.astype(f16)[None, :]                       # [1, 40]
    eye = np.eye(OUT, dtype=np.float32)

    # x -> per-core [H+1, T, BC] fp16 with ones channel at row H
    xs = x.reshape(NCORES, BC, T, H).astype(f16)
    in_maps = []
    for c in range(NCORES):
        xc = np.empty((H + 1, T, BC), dtype=f16)
        xc[0:H] = xs[c].transpose(2, 1, 0)                   # [H, T, BC]
        xc[H] = 1.0
        in_maps.append(
            {
                "xT": np.ascontiguousarray(xc),
                "w1": w1a,
                "w2": w2t,
                "bout": boutq,
                "eye": eye,
            }
        )
    return in_maps


def kernel(x, W_ih, W_hh, b_ih, b_hh, W_out, b_out, _bench=None):
    x = np.asarray(x, dtype=np.float32)
    W_ih = np.asarray(W_ih, dtype=np.float32)
    b_ih = np.asarray(b_ih, dtype=np.float32)
    b_hh = np.asarray(b_hh, dtype=np.float32)
    W_out = np.asarray(W_out, dtype=np.float32)
    b_out = np.asarray(b_out, dtype=np.float32)

    from concourse.bass_utils import run_bass_kernel_spmd

    if "nc" not in _CACHE:
        _CACHE["nc"] = _build()
    nc = _CACHE["nc"]

    in_maps = _prep_inputs(x, W_ih, b_ih, b_hh, W_out, b_out)
    kwargs = dict(_bench) if _bench else {}
    res = run_bass_kernel_spmd(nc, in_maps, core_ids=list(range(NCORES)), **kwargs)
    out = np.concatenate([r["y"] for r in res.results], axis=0)  # [1024, 40]
    if _bench is not None:
        _CACHE["last_result"] = res
    return out.reshape(B, 4, 10).astype(np.float32)
